# revision 2
# baseline (speedup 1.0000x reference)
"""Trainium2 Bass kernel for the CRU (continuous recurrent unit / time-decay GRU).

Model (per batch element b, sequence step t):
    decay  = exp(-softplus(dt_t * w_decay + b_decay)) = sigmoid(-(dt_t * w_decay + b_decay))
    h      = h * decay                     (skipped at t=0; h0 = 0 so it's a no-op anyway)
    gi     = [v_t, m_t] @ W_ih.T + b_ih    (3H)
    gh     = h @ W_hh.T + b_hh             (3H)
    r      = sigmoid(gi_r + gh_r)
    z      = sigmoid(gi_z + gh_z)
    n      = tanh(gi_n + r * gh_n)
    h      = (1 - z) * n + z * h
    out    = h_T @ W_head.T + b_head

Sharding: data-parallel over batch B=2048 across 8 NeuronCores (256/core).
Layout on device is "transposed": h^T is [H, B_loc] with the H axis split into
4 partition-chunks of 128, kept as one SBUF tile [128, 4, B_loc].  All matmuls
are out[gate_chunk, batch] = W_chunk.T.T @ h^T with weights stationary.
Biases are applied for free via the ScalarE activation per-partition bias.
The decay is one ACT op per chunk: sigmoid(scale_p * dt_b + bias_p) with the
per-partition scale = -w_decay.

dtype: fp16 for matmul operands and elementwise chain (DVE 2x mode), fp32 PSUM
accumulation and fp32 biases => final rel-err ~1e-3.
"""

import sys

if "/opt/trn_rl_repo" not in sys.path:
    sys.path.insert(0, "/opt/trn_rl_repo")

import numpy as np

import concourse.bacc as bacc
import concourse.tile as tile
import concourse.mybir as mybir
from concourse.bass_utils import run_bass_kernel_spmd

B, T, D, H, NT = 2048, 512, 64, 512, 8
IN = 2 * D  # 128
NCORES = 8
BL = B // NCORES  # 256
HK = H // 128  # 4 chunks of the hidden dim
SB = 4  # sequence steps per DMA block

F16 = mybir.dt.float16
F32 = mybir.dt.float32
ACTF = mybir.ActivationFunctionType
ALU = mybir.AluOpType


def build_program(t_steps=T):
    """Builds the SPMD 8-core bass program. Returns the compiled Bacc."""
    assert t_steps % SB == 0
    nblk = t_steps // SB

    nc = bacc.Bacc(
        "TRN2",
        target_bir_lowering=False,
        debug=False,
        enable_asserts=False,
        num_devices=NCORES,
    )

    # ---- DRAM I/O ----
    inp_d = nc.dram_tensor("inp", [nblk, 128, SB, BL], F16, kind="ExternalInput").ap()
    dt_d = nc.dram_tensor("dts", [nblk, SB, BL], F16, kind="ExternalInput").ap()
    wih_d = nc.dram_tensor("wih", [128, 12, 128], F16, kind="ExternalInput").ap()
    whh_d = nc.dram_tensor("whh", [128, HK, 12, 128], F16, kind="ExternalInput").ap()
    whd_d = nc.dram_tensor("whd", [128, HK, NT], F16, kind="ExternalInput").ap()
    brz_d = nc.dram_tensor("brz", [128, 8], F32, kind="ExternalInput").ap()
    bin_d = nc.dram_tensor("bin", [128, HK], F32, kind="ExternalInput").ap()
    bhn_d = nc.dram_tensor("bhn", [128, HK], F32, kind="ExternalInput").ap()
    ndw_d = nc.dram_tensor("ndw", [128, HK], F32, kind="ExternalInput").ap()
    ndb_d = nc.dram_tensor("ndb", [128, HK], F32, kind="ExternalInput").ap()
    bhd_d = nc.dram_tensor("bhd", [NT, 1], F32, kind="ExternalInput").ap()
    y_d = nc.dram_tensor("y", [NT, BL], F32, kind="ExternalOutput").ap()

    with tile.TileContext(nc) as tc:
        with (
            tc.tile_pool(name="const", bufs=1) as const,
            tc.tile_pool(name="pio", bufs=3) as pio,
            tc.tile_pool(name="pdec", bufs=2) as pdec,
            tc.tile_pool(name="ph", bufs=3) as ph,
            tc.tile_pool(name="pg", bufs=2) as pg,
            tc.tile_pool(name="pps", bufs=1, space="PSUM") as pps,
        ):
            # ---- constants ----
            wih_s = const.tile([128, 12, 128], F16, name="wih_s")
            nc.sync.dma_start(out=wih_s, in_=wih_d)
            whh_s = const.tile([128, HK, 12, 128], F16, name="whh_s")
            nc.sync.dma_start(out=whh_s, in_=whh_d)
            whd_s = const.tile([128, HK, NT], F16, name="whd_s")
            nc.sync.dma_start(out=whd_s, in_=whd_d)
            brz_s = const.tile([128, 8], F32, name="brz_s")
            nc.sync.dma_start(out=brz_s, in_=brz_d)
            bin_s = const.tile([128, HK], F32, name="bin_s")
            nc.sync.dma_start(out=bin_s, in_=bin_d)
            bhn_s = const.tile([128, HK], F32, name="bhn_s")
            nc.sync.dma_start(out=bhn_s, in_=bhn_d)
            ndw_s = const.tile([128, HK], F32, name="ndw_s")
            nc.sync.dma_start(out=ndw_s, in_=ndw_d)
            ndb_s = const.tile([128, HK], F32, name="ndb_s")
            nc.sync.dma_start(out=ndb_s, in_=ndb_d)
            bhd_s = const.tile([NT, 1], F32, name="bhd_s")
            nc.sync.dma_start(out=bhd_s, in_=bhd_d)

            h0 = const.tile([128, HK, BL], F16, name="h0")
            nc.vector.memset(h0, 0.0)

            h_prev = h0
            inp_blk = None
            dtb_blk = None

            for t in range(t_steps):
                tb, s = divmod(t, SB)
                if s == 0:
                    inp_blk = pio.tile([128, SB, BL], F16, name="inp_blk")
                    nc.sync.dma_start(out=inp_blk, in_=inp_d[tb])
                    dtb_blk = pio.tile([128, SB, BL], F16, name="dtb_blk")
                    nc.sync.dma_start(
                        out=dtb_blk, in_=dt_d[tb].partition_broadcast(128)
                    )
                inp_t = inp_blk[:, s, :]
                dtb_t = dtb_blk[:, s, :]

                # decay_k[p, b] = sigmoid(-w[128k+p] * dt_b - b_decay[128k+p])
                dec = pdec.tile([128, HK, BL], F16, name="dec")
                for k in range(HK):
                    nc.scalar.activation(
                        out=dec[:, k, :],
                        in_=dtb_t,
                        func=ACTF.Sigmoid,
                        bias=ndb_s[:, k : k + 1],
                        scale=ndw_s[:, k : k + 1],
                    )

                # hdec = h_prev * decay
                hdec = ph.tile([128, HK, BL], F16, name="hdec")
                for k in range(HK):
                    nc.vector.tensor_tensor(
                        out=hdec[:, k, :],
                        in0=h_prev[:, k, :],
                        in1=dec[:, k, :],
                        op=ALU.mult,
                    )

                # ---- PSUM tiles (8 banks total, reused every step) ----
                rz = [
                    pps.tile([128, 2, BL], F32, name=f"rz{m}", tag=f"rz{m}")
                    for m in range(4)
                ]
                inps = [
                    pps.tile([128, 2, BL], F32, name=f"inps{j}", tag=f"inps{j}")
                    for j in range(2)
                ]
                hnps = [
                    pps.tile([128, 2, BL], F32, name=f"hnps{j}", tag=f"hnps{j}")
                    for j in range(2)
                ]

                # Matmuls.  HW gotcha: start=True clears the has_written bits
                # of the ENTIRE psum bank, so only the first matmul touching a
                # bank in this step may use start=True.  Later matmuls with
                # start=False overwrite where the bit is clear and accumulate
                # where it is set, which is exactly what we need for the bank
                # regions that belong to a different accumulation group.
                seen_banks = set()

                def mm(ps_slice, bank_key, w, rhs, last=False):
                    first = bank_key not in seen_banks
                    seen_banks.add(bank_key)
                    nc.tensor.matmul(ps_slice, w, rhs, start=first, stop=last)

                # gi matmuls (input side, no h dependency)
                for m in range(4):
                    mm(rz[m][:, 0, :], ("rz", m), wih_s[:, m, :], inp_t)
                    mm(rz[m][:, 1, :], ("rz", m), wih_s[:, 4 + m, :], inp_t)
                for k in range(HK):
                    mm(
                        inps[k // 2][:, k % 2, :],
                        ("in", k // 2),
                        wih_s[:, 8 + k, :],
                        inp_t,
                        last=(k % 2 == 1),
                    )

                # gh matmuls, k-major so they start as soon as hdec[:,k,:] is ready
                for k in range(HK):
                    hk = hdec[:, k, :]
                    for m in range(4):
                        mm(rz[m][:, 0, :], ("rz", m), whh_s[:, k, m, :], hk)
                        mm(
                            rz[m][:, 1, :],
                            ("rz", m),
                            whh_s[:, k, 4 + m, :],
                            hk,
                            last=(k == HK - 1),
                        )
                    for j in range(4):
                        mm(
                            hnps[j // 2][:, j % 2, :],
                            ("hn", j // 2),
                            whh_s[:, k, 8 + j, :],
                            hk,
                            last=(k == HK - 1 and j % 2 == 1),
                        )

                # gates
                r = pg.tile([128, HK, BL], F16, name="r")
                z = pg.tile([128, HK, BL], F16, name="z")
                for m in range(4):
                    nc.scalar.activation(
                        out=r[:, m, :],
                        in_=rz[m][:, 0, :],
                        func=ACTF.Sigmoid,
                        bias=brz_s[:, m : m + 1],
                    )
                    nc.scalar.activation(
                        out=z[:, m, :],
                        in_=rz[m][:, 1, :],
                        func=ACTF.Sigmoid,
                        bias=brz_s[:, 4 + m : 5 + m],
                    )

                hnb = pg.tile([128, HK, BL], F16, name="hnb")
                n_t = pg.tile([128, HK, BL], F16, name="n_t")
                h_new = ph.tile([128, HK, BL], F16, name="h_new")
                for k in range(HK):
                    # hnb = psum_hn + b_hn  (the gh part of the n gate, biased)
                    nc.vector.tensor_scalar(
                        out=hnb[:, k, :],
                        in0=hnps[k // 2][:, k % 2, :],
                        scalar1=bhn_s[:, k : k + 1],
                        scalar2=None,
                        op0=ALU.add,
                    )
                    # hnb := r * hnb
                    nc.vector.tensor_tensor(
                        out=hnb[:, k, :], in0=r[:, k, :], in1=hnb[:, k, :], op=ALU.mult
                    )
                    # hnb := psum_in + hnb
                    nc.vector.tensor_tensor(
                        out=hnb[:, k, :],
                        in0=inps[k // 2][:, k % 2, :],
                        in1=hnb[:, k, :],
                        op=ALU.add,
                    )
                    # n = tanh(hnb + b_in)
                    nc.scalar.activation(
                        out=n_t[:, k, :],
                        in_=hnb[:, k, :],
                        func=ACTF.Tanh,
                        bias=bin_s[:, k : k + 1],
                    )
                    # hdec := hdec - n
                    nc.vector.tensor_tensor(
                        out=hdec[:, k, :],
                        in0=hdec[:, k, :],
                        in1=n_t[:, k, :],
                        op=ALU.subtract,
                    )
                    # z := z * (hdec - n)
                    nc.vector.tensor_tensor(
                        out=z[:, k, :], in0=z[:, k, :], in1=hdec[:, k, :], op=ALU.mult
                    )
                    # h_new = n + z * (hdec - n)
                    nc.vector.tensor_tensor(
                        out=h_new[:, k, :],
                        in0=n_t[:, k, :],
                        in1=z[:, k, :],
                        op=ALU.add,
                    )

                h_prev = h_new

            # ---- head: y = W_head @ h_T + b_head  -> [NT, BL] ----
            hd_ps = pps.tile([NT, BL], F32, name="hd_ps", tag="rz0")
            for k in range(HK):
                nc.tensor.matmul(
                    hd_ps,
                    whd_s[:, k, :],
                    h_prev[:, k, :],
                    start=(k == 0),
                    stop=(k == HK - 1),
                )
            y_sb = pg.tile([NT, BL], F32, name="y_sb")
            nc.scalar.activation(
                out=y_sb, in_=hd_ps, func=ACTF.Identity, bias=bhd_s
            )
            nc.sync.dma_start(out=y_d, in_=y_sb)

    nc.compile()
    return nc


def prepare_inputs(
    values, mask, timestamps, W_ih, W_hh, b_ih, b_hh, W_decay, b_decay, W_head, b_head,
    t_steps=T,
):
    """Host-side reshaping into the per-core in_maps."""
    values = np.asarray(values, dtype=np.float32)
    mask = np.asarray(mask, dtype=np.float32)
    timestamps = np.asarray(timestamps, dtype=np.float32)
    W_ih = np.asarray(W_ih, dtype=np.float32)
    W_hh = np.asarray(W_hh, dtype=np.float32)
    b_ih = np.asarray(b_ih, dtype=np.float32)
    b_hh = np.asarray(b_hh, dtype=np.float32)
    W_decay = np.asarray(W_decay, dtype=np.float32)
    b_decay = np.asarray(b_decay, dtype=np.float32)
    W_head = np.asarray(W_head, dtype=np.float32)
    b_head = np.asarray(b_head, dtype=np.float32)

    nblk = t_steps // SB

    dt = np.zeros((B, T), dtype=np.float32)
    dt[:, 1:] = timestamps[:, 1:] - timestamps[:, :-1]

    # weights (shared by all cores)
    wih = np.ascontiguousarray(W_ih.T.reshape(128, 12, 128)).astype(np.float16)
    whh = np.ascontiguousarray(
        W_hh.T.reshape(HK, 128, 12, 128).transpose(1, 0, 2, 3)
    ).astype(np.float16)
    whd = np.ascontiguousarray(W_head.T.reshape(HK, 128, NT).transpose(1, 0, 2)).astype(
        np.float16
    )
    bsum = (b_ih + b_hh)[: 2 * H]
    brz = np.ascontiguousarray(bsum.reshape(8, 128).T).astype(np.float32)
    bin_ = np.ascontiguousarray(b_ih[2 * H :].reshape(HK, 128).T).astype(np.float32)
    bhn = np.ascontiguousarray(b_hh[2 * H :].reshape(HK, 128).T).astype(np.float32)
    ndw = np.ascontiguousarray((-W_decay[:, 0]).reshape(HK, 128).T).astype(np.float32)
    ndb = np.ascontiguousarray((-b_decay).reshape(HK, 128).T).astype(np.float32)
    bhd = b_head.reshape(NT, 1).astype(np.float32)

    in_maps = []
    for c in range(NCORES):
        sl = slice(c * BL, (c + 1) * BL)
        # [T, 128, BL] : inp[t, 0:64, b] = values[b, t, :], inp[t, 64:128, b] = mask
        v = values[sl, :t_steps].transpose(1, 2, 0)  # [T, 64, BL]
        m = mask[sl, :t_steps].transpose(1, 2, 0)
        inp = np.concatenate([v, m], axis=1)  # [T, 128, BL]
        inp = (
            inp.reshape(nblk, SB, 128, BL).transpose(0, 2, 1, 3).astype(np.float16)
        )  # [nblk, 128, SB, BL]
        dts = (
            dt[sl, :t_steps].T.reshape(nblk, SB, BL).astype(np.float16)
        )  # [nblk, SB, BL]
        in_maps.append(
            dict(
                inp=np.ascontiguousarray(inp),
                dts=np.ascontiguousarray(dts),
                wih=wih,
                whh=whh,
                whd=whd,
                brz=brz,
                bin=bin_,
                bhn=bhn,
                ndw=ndw,
                ndb=ndb,
                bhd=bhd,
            )
        )
    return in_maps


_CACHE = {}


def _get_program(t_steps=T):
    if t_steps not in _CACHE:
        _CACHE[t_steps] = build_program(t_steps)
    return _CACHE[t_steps]


def kernel(**inputs):
    nc = _get_program(T)
    in_maps = prepare_inputs(**inputs)
    res = run_bass_kernel_spmd(nc, in_maps, core_ids=list(range(NCORES)))
    outs = [r["y"].T for r in res.results]  # each [BL, NT]
    return np.ascontiguousarray(np.concatenate(outs, axis=0).astype(np.float32))
